# revision 37
# baseline (speedup 1.0000x reference)
"""Born-potential GNN message-passing kernel for 8 Trainium2 NeuronCores.

Strategy
--------
Host side (sharding / data staging only):
  * Edges are sorted by idx_i and grouped into 128-atom chunks; atoms are
    assigned to chunks by descending degree so every chunk has near-uniform
    degree (tight padding). Chunks are dealt to the 8 cores in octets so all
    cores see identical segment shapes (SPMD single program).
  * Within a segment, partition p holds exactly the edges of one atom, so all
    i-side per-atom quantities are per-partition scalars (no gather needed).
  * j-side per-atom scalars (|q_j| as bf16, ns_j/2 as u16 code, and the
    film/Z class code) are staged into the edge stream by the host (the
    hardware has no scalable fine-grained gather instrument).
Device side:
  * Builds the 16-bit quantized log-r0 pair table and performs the per-edge
    pair-table lookup with the GPSIMD ap_gather instruction.
  * All per-edge arithmetic (distances, exponentials, Born potential,
    cutoff mask) on the vector/scalar engines.
  * Segment row-reduction to per-atom sums, then a one-hot matmul
    accumulating atoms into the 128 molecule bins in PSUM.
  * Output per core: [128] partial molecule energies; host sums the 8 parts.
"""

import sys

sys.path.insert(0, "/opt/trn_rl_repo")

import numpy as np

import concourse.bacc as bacc
import concourse.bass as bass
import concourse.mybir as mybir
import concourse.tile as tile
from concourse.bass_utils import run_bass_kernel_spmd

P = 128
NCORE = 8
KE = 14.3996
CUTOFF = 5.0
LN5 = float(np.log(CUTOFF))

# log-r0 16-bit quantization
R0_LO = float(np.log(0.25))
R0_HI = float(np.log(4.0))
R0_SC = 65500.0 / (R0_HI - R0_LO)
R0_DEC = 1.0 / R0_SC

# ns/2 - 3 in [0, 2) encoded as u16
NS_OFF = 3.0
NS_SC = 16383.75
NS_DEC = 1.0 / NS_SC

F32 = mybir.dt.float32
I32 = mybir.dt.int32
I16 = mybir.dt.int16


def _plan(idx_i, n_atoms):
    """Host-side layout plan: degree-balanced chunking + edge placement."""
    E = idx_i.shape[0]
    deg = np.bincount(idx_i, minlength=n_atoms).astype(np.int64)
    nchunk = -(-n_atoms // P)
    nchunk = -(-nchunk // NCORE) * NCORE          # pad to multiple of NCORE
    a_pad = nchunk * P
    deg_pad = np.zeros(a_pad, np.int64)
    deg_pad[:n_atoms] = deg
    order = np.argsort(-deg_pad, kind="stable")   # atom ids by degree desc
    pos = np.empty(a_pad, np.int64)
    pos[order] = np.arange(a_pad)

    nseg = nchunk // NCORE
    # rank r = ((s*8 + k)*128 + p)
    degmat = deg_pad[order].reshape(nseg, NCORE, P)
    lseg = degmat.max(axis=(1, 2))
    lseg = np.maximum((lseg + 7) // 8 * 8, 8).astype(np.int64)
    coloff = np.zeros(nseg + 1, np.int64)
    coloff[1:] = np.cumsum(lseg)
    ltot = int(coloff[-1])

    # per-edge placement
    perm = np.argsort(idx_i, kind="stable")
    a_sorted = idx_i[perm].astype(np.int64)
    start = np.zeros(n_atoms + 1, np.int64)
    np.cumsum(deg, out=start[1:])
    rank = np.arange(E, dtype=np.int64) - start[a_sorted]
    pos_e = pos[a_sorted]
    chunk_e = pos_e >> 7
    core_e = chunk_e & 7
    seg_e = chunk_e >> 3
    row_e = pos_e & 127
    col_e = coloff[seg_e] + rank

    atom_ids = order.reshape(nseg, NCORE, P).transpose(1, 2, 0)  # [k, p, s]
    return dict(
        a_pad=a_pad, nseg=nseg, lseg=lseg.tolist(), coloff=coloff, ltot=ltot,
        perm=perm, core_e=core_e, row_e=row_e, col_e=col_e, atom_ids=atom_ids,
    )


def _build_nc(nseg, lseg, coloff, ltot, r0_pad_len, q_dec, dbg=False):
    """Build the SPMD Bass program (identical on all cores)."""
    Q_DEC = float(q_dec)
    nc = bacc.Bacc("TRN2", target_bir_lowering=False, debug=True)
    dbg_outs = {}
    if dbg:
        L0 = lseg[0]
        for nm, shp, dt in [
            ("dbg_word", [P, L0], I32), ("dbg_sel", [P, L0], F32),
            ("dbg_pot", [P, L0], F32), ("dbg_ycol", [P, 1], F32),
            ("dbg_n", [P, L0], F32), ("dbg_logr0", [P, L0], F32),
            ("dbg_d2", [P, L0], F32), ("dbg_oh", [P, P], F32),
        ]:
            dbg_outs[nm] = nc.declare_dram_parameter(nm, shp, dt, isOutput=True)

    xs = nc.declare_dram_parameter("xs", [P, ltot], F32, isOutput=False)
    ys = nc.declare_dram_parameter("ys", [P, ltot], F32, isOutput=False)
    zs = nc.declare_dram_parameter("zs", [P, ltot], F32, isOutput=False)
    ji = nc.declare_dram_parameter("ji", [P, ltot], I32, isOutput=False)
    bc = nc.declare_dram_parameter("bc", [P, ltot], I32, isOutput=False)
    q_cols = nc.declare_dram_parameter("q_cols", [P, nseg], F32, isOutput=False)
    ns_cols = nc.declare_dram_parameter("ns_cols", [P, nseg], F32, isOutput=False)
    a_cols = nc.declare_dram_parameter("a_cols", [P, nseg], I32, isOutput=False)
    m_cols = nc.declare_dram_parameter("m_cols", [P, nseg], F32, isOutput=False)
    r0_flat = nc.declare_dram_parameter("r0_flat", [r0_pad_len], F32, isOutput=False)
    out = nc.declare_dram_parameter("out", [P, 1], F32, isOutput=True)

    npair = r0_pad_len // 2     # r0 pair words total (padded)
    wcols = npair // P          # pair words per partition
    NWORD = 16200               # real pair words

    r0code = nc.dram_tensor("r0code", [npair], I32)

    with tile.TileContext(nc) as tc:
        with (
            tc.tile_pool(name="setup", bufs=1) as sp,
            tc.tile_pool(name="edge", bufs=2) as ep,
            tc.tile_pool(name="psum", bufs=1, space="PSUM") as pp,
        ):
            A = mybir.AluOpType
            # ---- constants ----
            iota_i = sp.tile([P, P], I32)
            nc.gpsimd.iota(iota_i[:], pattern=[[1, P]], base=0, channel_multiplier=0)
            iota_f = sp.tile([P, P], F32)
            nc.vector.tensor_copy(iota_f[:], iota_i[:])

            # mask16[p, r] = (r == p % 16), as int32
            i16t = sp.tile([P, 16], I32)
            nc.gpsimd.iota(i16t[:], pattern=[[1, 16]], base=0, channel_multiplier=0)
            pid = sp.tile([P, 1], I32)
            nc.gpsimd.iota(pid[:], pattern=[[1, 1]], base=0, channel_multiplier=1)
            pmod = sp.tile([P, 1], I32)
            nc.vector.tensor_scalar(pmod[:], pid[:], 15, None, A.bitwise_and)
            i16f = sp.tile([P, 16], F32)
            nc.vector.tensor_copy(i16f[:], i16t[:])
            pmodf = sp.tile([P, 1], F32)
            nc.vector.tensor_copy(pmodf[:], pmod[:])
            mskf0 = sp.tile([P, 16], F32)
            nc.vector.tensor_scalar(mskf0[:], i16f[:], pmodf[:], None, A.is_equal)
            mskf = sp.tile([P, 1, 16], F32)
            nc.vector.tensor_copy(
                mskf[:], mskf0[:].rearrange("p (one r) -> p one r", one=1))

            # ---- r0 code table: 16-bit log-r0, packed in pairs ----
            r0t = sp.tile([P, wcols, 2], F32, tag="r0a")
            nc.sync.dma_start(
                out=r0t[:], in_=r0_flat[:].rearrange("(p c) -> p c", p=P))
            r0l = sp.tile([P, wcols, 2], F32, tag="r0b")
            nc.scalar.activation(r0l[:], r0t[:], mybir.ActivationFunctionType.Ln)
            r0q = sp.tile([P, wcols, 2], F32, tag="r0c")
            nc.scalar.activation(r0q[:], r0l[:], mybir.ActivationFunctionType.Copy,
                                 bias=float(-R0_LO * R0_SC + 0.5), scale=float(R0_SC))
            r0i = sp.tile([P, wcols, 2], I32, tag="r0d")
            nc.vector.tensor_copy(r0i[:], r0q[:])
            r0sh = sp.tile([P, wcols], I32, tag="r0f")
            nc.vector.tensor_scalar(
                r0sh[:], r0i[:, :, 1], 16, None, A.logical_shift_left)
            r0w = sp.tile([P, wcols], I32, tag="r0e")
            nc.vector.tensor_tensor(
                out=r0w[:], in0=r0sh[:], in1=r0i[:, :, 0], op=A.bitwise_or)
            nc.sync.dma_start(
                out=r0code[:].rearrange("(p c) -> p c", p=P), in_=r0w[:])

            # replicate the table to all partitions for ap_gather.
            # (barrier: the r0code DRAM round-trip is not dep-tracked)
            tc.strict_bb_all_engine_barrier()
            r0row = sp.tile([1, npair], I32, tag="r0g")
            nc.sync.dma_start(
                out=r0row[:], in_=r0code[:].rearrange("(one c) -> one c", one=1))
            r0rep = sp.tile([P, npair], I32, tag="r0rep")
            nc.gpsimd.partition_broadcast(r0rep[:], r0row[:], channels=P)

            # ---- per-partition atom columns ----
            qc = sp.tile([P, nseg], F32)
            nc.sync.dma_start(out=qc[:], in_=q_cols[:])
            qa = sp.tile([P, nseg], F32)
            nc.scalar.activation(qa[:], qc[:], mybir.ActivationFunctionType.Abs)
            nsc2 = sp.tile([P, nseg], F32)
            nc.sync.dma_start(out=nsc2[:], in_=ns_cols[:])
            ns3 = sp.tile([P, nseg], F32)
            nc.vector.tensor_scalar_add(ns3[:], nsc2[:], NS_OFF)
            ac = sp.tile([P, nseg], I32)
            nc.sync.dma_start(out=ac[:], in_=a_cols[:])
            mc = sp.tile([P, nseg], F32)
            nc.sync.dma_start(out=mc[:], in_=m_cols[:])

            psum = pp.tile([P, 1], F32, space="PSUM")

            # r0code DRAM round-trip is not dep-tracked by Tile — hard barrier.
            tc.strict_bb_all_engine_barrier()

            # ---- main loop over segments ----
            for s in range(nseg):
                L = lseg[s]
                off = int(coloff[s])

                xt = ep.tile([P, L], F32, tag="x")
                nc.sync.dma_start(out=xt[:], in_=xs[:, off:off + L])
                yt = ep.tile([P, L], F32, tag="y")
                nc.sync.dma_start(out=yt[:], in_=ys[:, off:off + L])
                zt2 = ep.tile([P, L], F32, tag="z")
                nc.sync.dma_start(out=zt2[:], in_=zs[:, off:off + L])
                jt = ep.tile([P, L], I32, tag="j")
                nc.sync.dma_start(out=jt[:], in_=ji[:, off:off + L])
                bt = ep.tile([P, L], I32, tag="b")
                nc.sync.dma_start(out=bt[:], in_=bc[:, off:off + L])

                # j unpack: hi16 = |q_j| u16 code, lo16 = ns code
                qj = ep.tile([P, L], I32, tag="qj")
                nc.vector.tensor_scalar(
                    qj[:], jt[:], 16, None, A.logical_shift_right)
                qjf = ep.tile([P, L], F32, tag="qjf")
                nc.vector.tensor_copy(qjf[:], qj[:])
                vt = ep.tile([P, L], I32, tag="vt")
                nc.vector.tensor_scalar(vt[:], jt[:], 0xFFFF, None, A.bitwise_and)
                vf = ep.tile([P, L], F32, tag="vf")
                nc.vector.tensor_copy(vf[:], vt[:])
                n = ep.tile([P, L], F32, tag="n")
                nc.vector.tensor_scalar(
                    n[:], vf[:], NS_DEC, ns3[:, s:s + 1], A.mult, A.add)

                # r0 pair lookup: widx = Acode_i + Bcode_j
                widx = ep.tile([P, L], I32, tag="widx")
                nc.vector.tensor_tensor(
                    out=widx[:], in0=bt[:],
                    in1=ac[:, s:s + 1].to_broadcast([P, L]), op=A.add)
                word = ep.tile([P, L], I32, tag="word")
                nc.vector.tensor_scalar(word[:], widx[:], 1, None, A.logical_shift_right)
                word16 = ep.tile([P, L], I16, tag="word16")
                nc.vector.tensor_copy(word16[:], word[:])
                shamt = ep.tile([P, L], I32, tag="shamt")
                nc.vector.tensor_scalar(
                    shamt[:], widx[:], 1, 4, A.bitwise_and, A.logical_shift_left)

                rga = ep.tile([P, 16 * L], I32, tag="rga")
                nc.gpsimd.ap_gather(
                    rga[:], r0rep[:, :NWORD], word16[:],
                    channels=P, num_elems=NWORD, d=1, num_idxs=16 * L)
                # extract the 16-bit code (exact bitwise ops), then select the
                # partition's own lane with a float one-hot reduce (≤ 65535,
                # exact in f32; DVE int mult/add are lossy above 24 bits).
                rsh = ep.tile([P, L, 16], I32, tag="rsh")
                nc.vector.tensor_tensor(
                    out=rsh[:], in0=rga[:].rearrange("p (c r) -> p c r", r=16),
                    in1=shamt[:].rearrange("p (c one) -> p c one", one=1)
                        .to_broadcast([P, L, 16]),
                    op=A.logical_shift_right)
                nc.vector.tensor_scalar(rsh[:], rsh[:], 0xFFFF, None, A.bitwise_and)
                rf = ep.tile([P, L, 16], F32, tag="rf")
                nc.vector.tensor_copy(rf[:], rsh[:])
                nc.vector.tensor_tensor(
                    out=rf[:], in0=rf[:],
                    in1=mskf[:].to_broadcast([P, L, 16]), op=A.mult)
                cf = ep.tile([P, L], F32, tag="cf")
                nc.vector.tensor_reduce(
                    cf[:], rf[:], axis=mybir.AxisListType.X, op=A.add)
                logr0 = ep.tile([P, L], F32, tag="logr0")
                nc.vector.tensor_scalar(
                    logr0[:], cf[:], R0_DEC, R0_LO, A.mult, A.add)

                # Born math
                nm1 = ep.tile([P, L], F32, tag="nm1")
                nc.vector.tensor_scalar_add(nm1[:], n[:], -1.0)
                t = ep.tile([P, L], F32, tag="t")
                nc.vector.tensor_mul(out=t[:], in0=nm1[:], in1=logr0[:])
                e1 = ep.tile([P, L], F32, tag="e1")
                nc.scalar.activation(e1[:], t[:], mybir.ActivationFunctionType.Exp)
                rn = ep.tile([P, L], F32, tag="rn")
                nc.vector.reciprocal(rn[:], n[:])
                qq = ep.tile([P, L], F32, tag="qq")
                nc.vector.tensor_scalar(
                    qq[:], qjf[:], Q_DEC, qa[:, s:s + 1], A.mult, A.mult)
                b1 = ep.tile([P, L], F32, tag="b1")
                nc.vector.tensor_mul(out=b1[:], in0=qq[:], in1=e1[:])
                bb = ep.tile([P, L], F32, tag="bb")
                nc.vector.tensor_mul(out=bb[:], in0=b1[:], in1=rn[:])

                sqx = ep.tile([P, L], F32, tag="sqx")
                nc.vector.tensor_mul(out=sqx[:], in0=xt[:], in1=xt[:])
                sqy = ep.tile([P, L], F32, tag="sqy")
                nc.vector.tensor_mul(out=sqy[:], in0=yt[:], in1=yt[:])
                s1 = ep.tile([P, L], F32, tag="s1")
                nc.vector.tensor_add(out=s1[:], in0=sqx[:], in1=sqy[:])
                sqz = ep.tile([P, L], F32, tag="sqz")
                nc.vector.tensor_mul(out=sqz[:], in0=zt2[:], in1=zt2[:])
                d2 = ep.tile([P, L], F32, tag="d2")
                nc.vector.tensor_add(out=d2[:], in0=s1[:], in1=sqz[:])

                l2 = ep.tile([P, L], F32, tag="l2")
                nc.scalar.activation(l2[:], d2[:], mybir.ActivationFunctionType.Ln)
                u = ep.tile([P, L], F32, tag="u")
                nc.vector.tensor_mul(out=u[:], in0=n[:], in1=l2[:])
                p1 = ep.tile([P, L], F32, tag="p1")
                nc.scalar.activation(p1[:], u[:], mybir.ActivationFunctionType.Exp,
                                     scale=-0.5)
                pc = ep.tile([P, L], F32, tag="pc")
                nc.scalar.activation(pc[:], n[:], mybir.ActivationFunctionType.Exp,
                                     scale=-LN5)
                diff = ep.tile([P, L], F32, tag="diff")
                nc.vector.tensor_sub(out=diff[:], in0=p1[:], in1=pc[:])
                pot = ep.tile([P, L], F32, tag="pot")
                nc.vector.tensor_mul(out=pot[:], in0=bb[:], in1=diff[:])

                ycol = ep.tile([P, 1], F32, tag="ycol")
                potm = ep.tile([P, L], F32, tag="potm")
                nc.vector.scalar_tensor_tensor(
                    potm[:], d2[:], float(CUTOFF * CUTOFF), pot[:],
                    A.is_le, A.mult, accum_out=ycol[:])

                # one-hot molecule binning
                oh = ep.tile([P, P], F32, tag="oh")
                nc.vector.tensor_scalar(
                    oh[:], iota_f[:], mc[:, s:s + 1], None, A.is_equal)
                nc.tensor.matmul(psum[:], lhsT=oh[:], rhs=ycol[:],
                                 start=(s == 0), stop=(s == nseg - 1))

                if dbg and s == 0:
                    nc.sync.dma_start(out=dbg_outs["dbg_word"][:], in_=word[:])
                    nc.sync.dma_start(out=dbg_outs["dbg_sel"][:], in_=cf[:])
                    nc.sync.dma_start(out=dbg_outs["dbg_pot"][:], in_=potm[:])
                    nc.sync.dma_start(out=dbg_outs["dbg_ycol"][:], in_=ycol[:])
                    nc.sync.dma_start(out=dbg_outs["dbg_n"][:], in_=n[:])
                    nc.sync.dma_start(out=dbg_outs["dbg_logr0"][:], in_=logr0[:])
                    nc.sync.dma_start(out=dbg_outs["dbg_d2"][:], in_=d2[:])
                    nc.sync.dma_start(out=dbg_outs["dbg_oh"][:], in_=oh[:])

            res = sp.tile([P, 1], F32)
            nc.scalar.activation(res[:], psum[:], mybir.ActivationFunctionType.Copy,
                                 scale=float(0.5 * KE))
            nc.sync.dma_start(out=out[:], in_=res[:])

    nc.finalize()
    return nc


def kernel(_dbg=False, _trace=False, **inputs):
    q = np.asarray(inputs["partial_charges"], np.float32)
    Z = np.asarray(inputs["Z"], np.int32)
    ns = np.asarray(inputs["ns"], np.float32)
    idx_m = np.asarray(inputs["idx_m"], np.int32)
    Rij = np.asarray(inputs["Rij"], np.float32)
    idx_i = np.asarray(inputs["idx_i"], np.int32)
    idx_j = np.asarray(inputs["idx_j"], np.int32)
    is_film = np.asarray(inputs["is_film"], np.int32)
    r0_table = np.asarray(inputs["r0_table"], np.float32)

    n_atoms = q.shape[0]
    plan = _plan(idx_i, n_atoms)
    a_pad, nseg, ltot = plan["a_pad"], plan["nseg"], plan["ltot"]

    def pad_atoms(v, fill, dtype):
        arr = np.full(a_pad, fill, dtype)
        arr[:n_atoms] = v
        return arr

    q_pad = pad_atoms(q, 0.0, np.float32)
    ns_pad = pad_atoms(ns, 8.0, np.float32)
    film_pad = pad_atoms(is_film, 0, np.int32)
    z_pad = pad_atoms(Z, 0, np.int32)
    m_pad = pad_atoms(idx_m, 127, np.int32)

    # per-edge j-side staging: |q_j| u16 code (hi) | u16 ns-code (lo); Bcode
    qabs = np.abs(q).astype(np.float64)
    qmax = max(float(qabs.max()), 1e-30)
    q_sc = 65535.0 / qmax
    q_dec = qmax / 65535.0
    qcode = np.clip(np.round(qabs * q_sc), 0, 65535).astype(np.uint32)
    nscode = np.clip(np.round((ns.astype(np.float64) * 0.5 - NS_OFF) * NS_SC),
                     0, 65535).astype(np.uint32)
    jinfo_atom = ((qcode << 16) | nscode).astype(np.int32)
    bcode_atom = (is_film * 8100 + Z).astype(np.int32)

    # r0 flat, padded so pair words fill whole partitions
    r0f = r0_table.reshape(-1)
    npair = -(-r0f.shape[0] // 2)
    npair = -(-npair // P) * P
    r0_pad = np.ones(npair * 2, np.float32)
    r0_pad[:r0f.shape[0]] = r0f

    perm, core_e, row_e, col_e = (plan["perm"], plan["core_e"], plan["row_e"],
                                  plan["col_e"])

    def place(vals, fill, dtype):
        arr = np.full((NCORE, P, ltot), fill, dtype)
        arr[core_e, row_e, col_e] = vals[perm]
        return arr

    xs = place(Rij[:, 0], 10.0, np.float32)
    ys = place(Rij[:, 1], 0.0, np.float32)
    zs = place(Rij[:, 2], 0.0, np.float32)
    ji = place(jinfo_atom[idx_j], jinfo_atom[0], np.int32)
    bc = place(bcode_atom[idx_j], 0, np.int32)

    aid = plan["atom_ids"]  # [k, p, s]
    q_cols = q_pad[aid]
    ns_cols = ns_pad[aid]
    a_cols = (film_pad[aid] * 16200 + z_pad[aid] * 90).astype(np.int32)
    m_cols = m_pad[aid].astype(np.float32)

    nc = _build_nc(nseg, plan["lseg"], plan["coloff"], ltot, npair * 2, q_dec,
                   dbg=_dbg)

    in_maps = []
    for k in range(NCORE):
        in_maps.append({
            "xs": xs[k], "ys": ys[k], "zs": zs[k], "ji": ji[k], "bc": bc[k],
            "q_cols": q_cols[k], "ns_cols": ns_cols[k],
            "a_cols": a_cols[k], "m_cols": m_cols[k],
            "r0_flat": r0_pad,
        })

    res = run_bass_kernel_spmd(nc, in_maps, list(range(NCORE)), trace=_trace)
    total = np.zeros(P, np.float64)
    for k in range(NCORE):
        total += res.results[k]["out"].reshape(P).astype(np.float64)
    if _trace and res.exec_time_ns is not None:
        print(f"HW exec time: {res.exec_time_ns} ns")
    if _dbg:
        return total.astype(np.float32), res, plan, in_maps
    return total.astype(np.float32)
